# revision 2
# baseline (speedup 1.0000x reference)
"""Trainium2 Bass kernel for nn_D_GCN (Chebyshev-style GCN diffusion).

Reference computation (per batch b):
    x0 = X                       (T, N, F) node features
    x1 = A x0                    (diffusion over nodes)
    x2 = 2 A x1 - x0
    out = relu(stack_k(x_k) @ Theta1 + bias)     Theta row index = f*K + k

Algebraic refactoring used here (Theta_k := Theta1[k::3]):
    out = relu( g0 + A @ (h1 + A @ h2) )
    g0  = x0 (Theta_0 - Theta_2) + bias    [host, f32]
    h1  = x0 Theta_1                       [host, f32]
    h2  = 2 x0 Theta_2                     [host, bf16]
All feature-dim matmuls (2% of FLOPs) are folded to host preprocessing;
the device runs the two dense N x N diffusion matmuls (98% of FLOPs,
all of A's traffic) as plain chained matmuls with no transposes.

Sharding: 8 cores = 2 batches x 4 node-blocks of 1024 rows.
Each core holds A^T[:, block] (bf16, SBUF-resident across both passes),
computes y = A_blk @ h2 for its rows, w = h1 + y, AllGathers w within
its 4-core batch group, then out_blk = relu(A_blk @ w + g0).

Pipelining: pass 1 runs column-block-major (4 blocks of 256 out rows),
so each w piece is ready early; 4 chunked AllGathers overlap the rest
of pass 1 and the start of pass 2 (pass 2 consumes gathered pieces in
order, so the last AllGather hides under the first 3 pieces' matmuls).
"""

import sys

if "/opt/trn_rl_repo" not in sys.path:
    sys.path.insert(0, "/opt/trn_rl_repo")

import numpy as np
import ml_dtypes

B, T, N, F, O = 2, 8, 4096, 32, 32
K = 3
NCORES = 8
NB = 4             # node blocks (shards) per batch
RS = N // NB       # rows per shard = 1024
NCH = RS // 128    # 8 n-chunks per shard
KC = N // 128      # 32 k-chunks (contraction)
TO = T * O         # 256 free columns
CB = 4             # column blocks (pieces) per shard
CBW = RS // CB     # 256 rows per piece = 2 n-chunks

_CACHE = {}


def _build_nc():
    import concourse.mybir as mybir
    import concourse.tile as tile
    from concourse import bacc

    f32 = mybir.dt.float32
    bf16 = mybir.dt.bfloat16

    nc = bacc.Bacc(None, num_devices=NCORES)

    # A^T shard, column(out-row)-block major: [cb, k, part, 256]
    AT_d = nc.dram_tensor("AT", [CB, KC, 128, CBW], bf16, kind="ExternalInput")
    H2_d = nc.dram_tensor("H2", [KC, 128, TO], bf16, kind="ExternalInput")
    H1_d = nc.dram_tensor("H1", [NCH, 128, TO], f32, kind="ExternalInput")
    G0_d = nc.dram_tensor("G0", [NCH, 128, TO], f32, kind="ExternalInput")
    OUT_d = nc.dram_tensor("OUT", [NCH, 128, TO], f32, kind="ExternalOutput")

    RG = [[0, 1, 2, 3], [4, 5, 6, 7]]

    with tile.TileContext(nc) as tc:
        with (
            tc.tile_pool(name="big", bufs=1) as big,
            tc.tile_pool(name="ps", bufs=1, space="PSUM") as psp,
            tc.tile_pool(name="dram", bufs=1, space="DRAM") as dram,
        ):
            AT = big.tile([128, CB, KC, CBW], bf16, name="ATs", tag="ATs")
            H2 = big.tile([128, KC, TO], bf16, name="H2s", tag="H2s")
            # gathered w, piece-major: slot (p, s) = global k-chunk j*8+p*2+q
            # with s = j*2 + q
            W = big.tile([128, CB, 2 * NB, TO], bf16, name="Ws", tag="Ws")
            H1 = big.tile([128, NCH, TO], f32, name="H1s", tag="H1s")
            G0 = big.tile([128, NCH, TO], f32, name="G0s", tag="G0s")
            WS = big.tile([128, NCH, TO], bf16, name="WSs", tag="WSs")
            OS = big.tile([128, NCH, TO], f32, name="OSs", tag="OSs")

            w_in = [dram.tile([CBW, TO], bf16, name=f"w_in{p}", tag=f"w_in{p}")
                    for p in range(CB)]
            w_all = [dram.tile([NB * CBW, TO], bf16, name=f"w_all{p}",
                               tag=f"w_all{p}") for p in range(CB)]

            # ---- input DMA streams on the SP ring (no waits -> FIFO ok) ----
            # first col-block in small chunks for a fast first matmul
            nc.sync.dma_start(
                H2[:, 0:8], H2_d[0:8].rearrange("k p n -> p k n"))
            nc.sync.dma_start(
                AT[:, 0, 0:8], AT_d[0, 0:8].rearrange("k p n -> p k n"))
            nc.sync.dma_start(
                H2[:, 8:16], H2_d[8:16].rearrange("k p n -> p k n"))
            nc.sync.dma_start(
                AT[:, 0, 8:16], AT_d[0, 8:16].rearrange("k p n -> p k n"))
            nc.sync.dma_start(
                H2[:, 16:24], H2_d[16:24].rearrange("k p n -> p k n"))
            nc.sync.dma_start(
                AT[:, 0, 16:24], AT_d[0, 16:24].rearrange("k p n -> p k n"))
            nc.sync.dma_start(
                H2[:, 24:32], H2_d[24:32].rearrange("k p n -> p k n"))
            nc.sync.dma_start(
                AT[:, 0, 24:32], AT_d[0, 24:32].rearrange("k p n -> p k n"))
            nc.sync.dma_start(
                H1[:, 0:2], H1_d[0:2].rearrange("k p n -> p k n"))
            for cb in range(1, CB):
                nc.sync.dma_start(
                    AT[:, cb, 0:16],
                    AT_d[cb, 0:16].rearrange("k p n -> p k n"))
                nc.sync.dma_start(
                    AT[:, cb, 16:32],
                    AT_d[cb, 16:32].rearrange("k p n -> p k n"))
                nc.sync.dma_start(
                    H1[:, 2 * cb:2 * cb + 2],
                    H1_d[2 * cb:2 * cb + 2].rearrange("k p n -> p k n"))
            # G0 + gathered-w loads ride the ACT ring (so a W load waiting on
            # a collective can't head-block later input loads)
            nc.scalar.dma_start(G0[:], G0_d[:].rearrange("k p n -> p k n"))

            # ---- pass 1 (col-block-major) + chunked AllGather pipeline ----
            ps1 = [psp.tile([128, TO], f32, name=f"y{n}", tag=f"bank{n}")
                   for n in range(NCH)]
            for cb in range(CB):
                with nc.named_scope(f"p1cb{cb}"):
                    for k in range(KC):
                        for i in range(2):
                            n = 2 * cb + i
                            nc.tensor.matmul(
                                ps1[n][:],
                                AT[:, cb, k, i * 128:(i + 1) * 128],
                                H2[:, k],
                                start=(k == 0),
                                stop=(k == KC - 1),
                            )
                    for i in range(2):
                        n = 2 * cb + i
                        nc.vector.tensor_add(WS[:, n], ps1[n][:], H1[:, n])
                    nc.sync.dma_start(
                        w_in[cb][:].rearrange("(c p) n -> p c n", p=128),
                        WS[:, 2 * cb:2 * cb + 2])
                    nc.gpsimd.collective_compute(
                        "AllGather",
                        mybir.AluOpType.bypass,
                        replica_groups=RG,
                        ins=[w_in[cb][:]],
                        outs=[w_all[cb][:]],
                    )
                    nc.scalar.dma_start(
                        W[:, cb],
                        w_all[cb][:].rearrange("(s p) n -> p s n", p=128))

            # ---- pass 2: accumulate pieces in gather order ----
            # piece p slot s=(j*2+q) holds global k-chunk j*8 + p*2 + q;
            # lhsT for out n-chunk n lives at AT[:, n//2, k, (n%2)*128:...]
            ps2 = [psp.tile([128, TO], f32, name=f"o{n}", tag=f"bank{n}")
                   for n in range(NCH)]

            def lhsT(kg, n):
                return AT[:, n // 2, kg, (n % 2) * 128:(n % 2 + 1) * 128]

            for p in range(CB - 1):
                with nc.named_scope(f"p2p{p}"):
                    for s in range(2 * NB):
                        kg = (s // 2) * (KC // NB) + p * 2 + (s % 2)
                        for n in range(NCH):
                            nc.tensor.matmul(
                                ps2[n][:], lhsT(kg, n), W[:, p, s],
                                start=(p == 0 and s == 0), stop=False)
            # last piece n-outer so each out chunk finishes early ->
            # epilogue (add g0, relu, store) pipelines with remaining matmuls
            p = CB - 1
            Relu = mybir.ActivationFunctionType.Relu
            with nc.named_scope("p2last"):
                for n in range(NCH):
                    for s in range(2 * NB):
                        kg = (s // 2) * (KC // NB) + p * 2 + (s % 2)
                        nc.tensor.matmul(
                            ps2[n][:], lhsT(kg, n), W[:, p, s],
                            start=False, stop=(s == 2 * NB - 1))
                    nc.vector.tensor_add(OS[:, n], ps2[n][:], G0[:, n])
                    nc.scalar.activation(OS[:, n], OS[:, n], Relu)
                    if n % 2 == 1:
                        nc.sync.dma_start(
                            OUT_d[n - 1:n + 1].rearrange("c p n -> p c n"),
                            OS[:, n - 1:n + 1])

    nc.compile()
    return nc


def _get_nc():
    if "nc" not in _CACHE:
        _CACHE["nc"] = _build_nc()
    return _CACHE["nc"]


def _prepare_in_maps(X, A_q, Theta1, bias):
    bf16 = ml_dtypes.bfloat16
    X = np.asarray(X, dtype=np.float32)
    A_q = np.asarray(A_q, dtype=np.float32)
    Theta1 = np.asarray(Theta1, dtype=np.float32)
    bias = np.asarray(bias, dtype=np.float32)

    Th = Theta1.reshape(F, K, O)
    Th0, Th1, Th2 = Th[:, 0], Th[:, 1], Th[:, 2]

    in_maps = []
    for b in range(B):
        Xb = X[b]                                   # (T, N, F)
        # [n, (t, o)] node-major layouts
        h2 = np.transpose(2.0 * (Xb @ Th2), (1, 0, 2)).reshape(N, TO)
        h1 = np.transpose(Xb @ Th1, (1, 0, 2)).reshape(N, TO)
        g0 = np.transpose(Xb @ (Th0 - Th2) + bias, (1, 0, 2)).reshape(N, TO)
        h2b = np.ascontiguousarray(h2).astype(bf16).reshape(KC, 128, TO)
        for j in range(NB):
            rows = slice(j * RS, (j + 1) * RS)
            # (4096, 1024) -> [cb, k, part, 256]
            ATj = A_q[b, rows].T.astype(bf16)
            ATj = np.ascontiguousarray(
                ATj.reshape(KC, 128, CB, CBW).transpose(2, 0, 1, 3))
            in_maps.append({
                "AT": ATj,
                "H2": h2b,
                "H1": np.ascontiguousarray(h1[rows]).reshape(NCH, 128, TO),
                "G0": np.ascontiguousarray(g0[rows]).reshape(NCH, 128, TO),
            })
    return in_maps


def run_with_results(inputs, **spmd_kwargs):
    """Returns (full_output, BassKernelResults). spmd_kwargs forwarded to
    run_bass_kernel_spmd (e.g. trace=True)."""
    from concourse.bass_utils import run_bass_kernel_spmd

    nc = _get_nc()
    in_maps = _prepare_in_maps(**inputs)
    res = run_bass_kernel_spmd(
        nc, in_maps, core_ids=list(range(NCORES)), **spmd_kwargs)

    out = np.empty((B, T, N, O), dtype=np.float32)
    for c in range(NCORES):
        b, j = divmod(c, NB)
        blk = res.results[c]["OUT"].reshape(RS, T, O)   # [n, t, o]
        out[b, :, j * RS:(j + 1) * RS, :] = np.transpose(blk, (1, 0, 2))
    return out, res


def kernel(X, A_q, Theta1, bias):
    out, _ = run_with_results(
        {"X": X, "A_q": A_q, "Theta1": Theta1, "bias": bias})
    return out


# revision 4
# speedup vs baseline: 1.1810x; 1.1810x over previous
"""Trainium2 Bass kernel for nn_D_GCN (Chebyshev-style GCN diffusion).

Reference computation (per batch b):
    x0 = X                       (T, N, F) node features
    x1 = A x0                    (diffusion over nodes)
    x2 = 2 A x1 - x0
    out = relu(stack_k(x_k) @ Theta1 + bias)     Theta row index = f*K + k

Algebraic refactoring used here (Theta_k := Theta1[k::3]):
    out = relu( g0 + A @ (h1 + A @ h2) )
    g0  = x0 (Theta_0 - Theta_2) + bias    [host, f32]
    h1  = x0 Theta_1                       [host, f32]
    h2  = 2 x0 Theta_2                     [host, bf16]
All feature-dim matmuls (2% of FLOPs) are folded to host preprocessing;
the device runs the two dense N x N diffusion matmuls (98% of FLOPs,
all of A's traffic) as plain chained matmuls with no transposes.

Sharding: 8 cores = 2 batches x 4 node-blocks of 1024 rows.
Each core holds A^T[:, block] (bf16, SBUF-resident across both passes),
computes y = A_blk @ h2 for its rows, w = h1 + y, AllGathers w within
its 4-core batch group, then out_blk = relu(A_blk @ w + g0).

Pipelining: pass 1 runs column-block-major (4 blocks of 256 out rows),
so each w piece is ready early; 4 chunked AllGathers overlap the rest
of pass 1 and the start of pass 2 (pass 2 consumes gathered pieces in
order, so the last AllGather hides under the first 3 pieces' matmuls).
"""

import sys

if "/opt/trn_rl_repo" not in sys.path:
    sys.path.insert(0, "/opt/trn_rl_repo")

import numpy as np
import ml_dtypes

B, T, N, F, O = 2, 8, 4096, 32, 32
K = 3
NCORES = 8
NB = 4             # node blocks (shards) per batch
RS = N // NB       # rows per shard = 1024
NCH = RS // 128    # 8 n-chunks per shard
KC = N // 128      # 32 k-chunks (contraction)
TO = T * O         # 256 free columns
CB = 4             # column blocks (pieces) per shard
CBW = RS // CB     # 256 rows per piece = 2 n-chunks

_CACHE = {}


def _build_nc():
    import concourse.mybir as mybir
    import concourse.tile as tile
    from concourse import bacc

    f32 = mybir.dt.float32
    bf16 = mybir.dt.bfloat16

    nc = bacc.Bacc(None, num_devices=NCORES)

    # A^T shard, column(out-row)-block major: [cb, k, part, 256]
    AT_d = nc.dram_tensor("AT", [CB, KC, 128, CBW], bf16, kind="ExternalInput")
    H2_d = nc.dram_tensor("H2", [KC, 128, TO], bf16, kind="ExternalInput")
    H1_d = nc.dram_tensor("H1", [NCH, 128, TO], f32, kind="ExternalInput")
    G0_d = nc.dram_tensor("G0", [NCH, 128, TO], f32, kind="ExternalInput")
    OUT_d = nc.dram_tensor("OUT", [NCH, 128, TO], f32, kind="ExternalOutput")

    RG = [[0, 1, 2, 3], [4, 5, 6, 7]]

    with tile.TileContext(nc) as tc:
        with (
            tc.tile_pool(name="big", bufs=1) as big,
            tc.tile_pool(name="ps", bufs=1, space="PSUM") as psp,
            tc.tile_pool(name="dram", bufs=1, space="DRAM") as dram,
        ):
            AT = big.tile([128, CB, KC, CBW], bf16, name="ATs", tag="ATs")
            H2 = big.tile([128, KC, TO], bf16, name="H2s", tag="H2s")
            # gathered w, piece-major: slot (p, s) = global k-chunk j*8+p*2+q
            # with s = j*2 + q
            W = big.tile([128, CB, 2 * NB, TO], bf16, name="Ws", tag="Ws")
            H1 = big.tile([128, NCH, TO], f32, name="H1s", tag="H1s")
            G0 = big.tile([128, NCH, TO], f32, name="G0s", tag="G0s")
            WS = big.tile([128, NCH, TO], bf16, name="WSs", tag="WSs")
            OS = big.tile([128, NCH, TO], f32, name="OSs", tag="OSs")

            w_in = [dram.tile([CBW, TO], bf16, name=f"w_in{p}", tag=f"w_in{p}")
                    for p in range(CB)]
            w_all = [dram.tile([NB * CBW, TO], bf16, name=f"w_all{p}",
                               tag=f"w_all{p}") for p in range(CB)]
            dummy_in = dram.tile([8, 4], f32, name="dummy_in", tag="dummy_in")
            dummy_out = dram.tile([64, 4], f32, name="dummy_out",
                                  tag="dummy_out")

            # Fire a tiny AllGather immediately: the runtime's one-time
            # first-collective barrier (~35 us) then overlaps the input DMA
            # ramp + pass 1 instead of gating the first real w gather.
            nc.gpsimd.collective_compute(
                "AllGather",
                mybir.AluOpType.bypass,
                replica_groups=[list(range(NCORES))],
                ins=[dummy_in[:]],
                outs=[dummy_out[:]],
            )

            # ---- input DMA streams on the SP ring (no waits -> FIFO ok) ----
            # first col-block in small chunks for a fast first matmul
            nc.sync.dma_start(
                H2[:, 0:8], H2_d[0:8].rearrange("k p n -> p k n"))
            nc.sync.dma_start(
                AT[:, 0, 0:8], AT_d[0, 0:8].rearrange("k p n -> p k n"))
            nc.sync.dma_start(
                H2[:, 8:16], H2_d[8:16].rearrange("k p n -> p k n"))
            nc.sync.dma_start(
                AT[:, 0, 8:16], AT_d[0, 8:16].rearrange("k p n -> p k n"))
            nc.sync.dma_start(
                H2[:, 16:24], H2_d[16:24].rearrange("k p n -> p k n"))
            nc.sync.dma_start(
                AT[:, 0, 16:24], AT_d[0, 16:24].rearrange("k p n -> p k n"))
            nc.sync.dma_start(
                H2[:, 24:32], H2_d[24:32].rearrange("k p n -> p k n"))
            nc.sync.dma_start(
                AT[:, 0, 24:32], AT_d[0, 24:32].rearrange("k p n -> p k n"))
            nc.sync.dma_start(
                H1[:, 0:2], H1_d[0:2].rearrange("k p n -> p k n"))
            for cb in range(1, CB):
                nc.sync.dma_start(
                    AT[:, cb, 0:16],
                    AT_d[cb, 0:16].rearrange("k p n -> p k n"))
                nc.sync.dma_start(
                    AT[:, cb, 16:32],
                    AT_d[cb, 16:32].rearrange("k p n -> p k n"))
                nc.sync.dma_start(
                    H1[:, 2 * cb:2 * cb + 2],
                    H1_d[2 * cb:2 * cb + 2].rearrange("k p n -> p k n"))
            # G0 + gathered-w loads ride the ACT ring (so a W load waiting on
            # a collective can't head-block later input loads)
            nc.scalar.dma_start(G0[:], G0_d[:].rearrange("k p n -> p k n"))

            # ---- pass 1 (col-block-major) + chunked AllGather pipeline ----
            ps1 = [psp.tile([128, TO], f32, name=f"y{n}", tag=f"bank{n}")
                   for n in range(NCH)]
            for cb in range(CB):
                with nc.named_scope(f"p1cb{cb}"):
                    for k in range(KC):
                        for i in range(2):
                            n = 2 * cb + i
                            nc.tensor.matmul(
                                ps1[n][:],
                                AT[:, cb, k, i * 128:(i + 1) * 128],
                                H2[:, k],
                                start=(k == 0),
                                stop=(k == KC - 1),
                            )
                    for i in range(2):
                        n = 2 * cb + i
                        nc.vector.tensor_add(WS[:, n], ps1[n][:], H1[:, n])
                    # SWDGE: keeps the w store off the input-load HWDGE
                    # rings (FIFO head-of-line) and on the same engine that
                    # triggers the collective.
                    nc.gpsimd.dma_start(
                        w_in[cb][:].rearrange("(c p) n -> p c n", p=128),
                        WS[:, 2 * cb:2 * cb + 2])
                    nc.gpsimd.collective_compute(
                        "AllGather",
                        mybir.AluOpType.bypass,
                        replica_groups=RG,
                        ins=[w_in[cb][:]],
                        outs=[w_all[cb][:]],
                    )
                    nc.scalar.dma_start(
                        W[:, cb],
                        w_all[cb][:].rearrange("(s p) n -> p s n", p=128))

            # ---- pass 2: accumulate pieces in gather order ----
            # piece p slot s=(j*2+q) holds global k-chunk j*8 + p*2 + q;
            # lhsT for out n-chunk n lives at AT[:, n//2, k, (n%2)*128:...]
            ps2 = [psp.tile([128, TO], f32, name=f"o{n}", tag=f"bank{n}")
                   for n in range(NCH)]

            def lhsT(kg, n):
                return AT[:, n // 2, kg, (n % 2) * 128:(n % 2 + 1) * 128]

            for p in range(CB - 1):
                with nc.named_scope(f"p2p{p}"):
                    for s in range(2 * NB):
                        kg = (s // 2) * (KC // NB) + p * 2 + (s % 2)
                        for n in range(NCH):
                            nc.tensor.matmul(
                                ps2[n][:], lhsT(kg, n), W[:, p, s],
                                start=(p == 0 and s == 0), stop=False)
            # last piece n-outer so each out chunk finishes early ->
            # epilogue (add g0, relu, store) pipelines with remaining matmuls
            p = CB - 1
            Relu = mybir.ActivationFunctionType.Relu
            with nc.named_scope("p2last"):
                for n in range(NCH):
                    for s in range(2 * NB):
                        kg = (s // 2) * (KC // NB) + p * 2 + (s % 2)
                        nc.tensor.matmul(
                            ps2[n][:], lhsT(kg, n), W[:, p, s],
                            start=False, stop=(s == 2 * NB - 1))
                    nc.vector.tensor_add(OS[:, n], ps2[n][:], G0[:, n])
                    nc.scalar.activation(OS[:, n], OS[:, n], Relu)
                    if n % 2 == 1:
                        nc.sync.dma_start(
                            OUT_d[n - 1:n + 1].rearrange("c p n -> p c n"),
                            OS[:, n - 1:n + 1])

    nc.compile()
    return nc


def _get_nc():
    if "nc" not in _CACHE:
        _CACHE["nc"] = _build_nc()
    return _CACHE["nc"]


def _prepare_in_maps(X, A_q, Theta1, bias):
    bf16 = ml_dtypes.bfloat16
    X = np.asarray(X, dtype=np.float32)
    A_q = np.asarray(A_q, dtype=np.float32)
    Theta1 = np.asarray(Theta1, dtype=np.float32)
    bias = np.asarray(bias, dtype=np.float32)

    Th = Theta1.reshape(F, K, O)
    Th0, Th1, Th2 = Th[:, 0], Th[:, 1], Th[:, 2]

    in_maps = []
    for b in range(B):
        Xb = X[b]                                   # (T, N, F)
        # [n, (t, o)] node-major layouts
        h2 = np.transpose(2.0 * (Xb @ Th2), (1, 0, 2)).reshape(N, TO)
        h1 = np.transpose(Xb @ Th1, (1, 0, 2)).reshape(N, TO)
        g0 = np.transpose(Xb @ (Th0 - Th2) + bias, (1, 0, 2)).reshape(N, TO)
        h2b = np.ascontiguousarray(h2).astype(bf16).reshape(KC, 128, TO)
        for j in range(NB):
            rows = slice(j * RS, (j + 1) * RS)
            # (4096, 1024) -> [cb, k, part, 256]
            ATj = A_q[b, rows].T.astype(bf16)
            ATj = np.ascontiguousarray(
                ATj.reshape(KC, 128, CB, CBW).transpose(2, 0, 1, 3))
            in_maps.append({
                "AT": ATj,
                "H2": h2b,
                "H1": np.ascontiguousarray(h1[rows]).reshape(NCH, 128, TO),
                "G0": np.ascontiguousarray(g0[rows]).reshape(NCH, 128, TO),
            })
    return in_maps


def run_with_results(inputs, **spmd_kwargs):
    """Returns (full_output, BassKernelResults). spmd_kwargs forwarded to
    run_bass_kernel_spmd (e.g. trace=True)."""
    from concourse.bass_utils import run_bass_kernel_spmd

    nc = _get_nc()
    in_maps = _prepare_in_maps(**inputs)
    res = run_bass_kernel_spmd(
        nc, in_maps, core_ids=list(range(NCORES)), **spmd_kwargs)

    out = np.empty((B, T, N, O), dtype=np.float32)
    for c in range(NCORES):
        b, j = divmod(c, NB)
        blk = res.results[c]["OUT"].reshape(RS, T, O)   # [n, t, o]
        out[b, :, j * RS:(j + 1) * RS, :] = np.transpose(blk, (1, 0, 2))
    return out, res


def kernel(X, A_q, Theta1, bias):
    out, _ = run_with_results(
        {"X": X, "A_q": A_q, "Theta1": Theta1, "bias": bias})
    return out


# revision 5
# speedup vs baseline: 1.2552x; 1.0629x over previous
"""Trainium2 Bass kernel for nn_D_GCN (Chebyshev-style GCN diffusion).

Reference computation (per batch b):
    x0 = X                       (T, N, F) node features
    x1 = A x0                    (diffusion over nodes)
    x2 = 2 A x1 - x0
    out = relu(stack_k(x_k) @ Theta1 + bias)     Theta row index = f*K + k

Algebraic refactoring used here (Theta_k := Theta1[k::3]):
    out = relu( g0 + A @ (h1 + A @ h2) )
    g0  = x0 (Theta_0 - Theta_2) + bias    [host, f32]
    h1  = x0 Theta_1                       [host, f32]
    h2  = 2 x0 Theta_2                     [host, bf16]
All feature-dim matmuls (2% of FLOPs) are folded to host preprocessing;
the device runs the two dense N x N diffusion matmuls (98% of FLOPs,
all of A's traffic) as plain chained matmuls with no transposes.

Sharding: 8 cores = 2 batches x 4 node-blocks of 1024 rows.
Each core holds A^T[:, block] (bf16, SBUF-resident across both passes),
computes y = A_blk @ h2 for its rows, w = h1 + y, AllGathers w within
its 4-core batch group, then out_blk = relu(A_blk @ w + g0).

Pipelining: pass 1 runs column-block-major (4 blocks of 256 out rows),
w is AllGathered in 2 pieces as soon as each half is ready, and pass 2
consumes the gathered pieces in order so the second gather overlaps the
first piece's matmuls. All dense inputs are laid out partition-major on
the host so each DMA moves large contiguous per-partition blocks.
"""

import sys

if "/opt/trn_rl_repo" not in sys.path:
    sys.path.insert(0, "/opt/trn_rl_repo")

import numpy as np
import ml_dtypes

B, T, N, F, O = 2, 8, 4096, 32, 32
K = 3
NCORES = 8
NB = 4             # node blocks (shards) per batch
RS = N // NB       # rows per shard = 1024
NCH = RS // 128    # 8 n-chunks per shard
KC = N // 128      # 32 k-chunks (contraction)
TO = T * O         # 256 free columns
CB = 4             # pass-1 column blocks per shard (256 out rows each)
CBW = RS // CB
NP = 2             # AllGather pieces (2 column blocks each)

_CACHE = {}


def _build_nc():
    import concourse.mybir as mybir
    import concourse.tile as tile
    from concourse import bacc

    f32 = mybir.dt.float32
    bf16 = mybir.dt.bfloat16

    nc = bacc.Bacc(None, num_devices=NCORES)

    # all inputs partition-major: leading dim 128 = SBUF partition
    AT_d = nc.dram_tensor("AT", [128, CB, KC, CBW], bf16, kind="ExternalInput")
    H2_d = nc.dram_tensor("H2", [128, KC, TO], bf16, kind="ExternalInput")
    H1_d = nc.dram_tensor("H1", [128, NCH, TO], f32, kind="ExternalInput")
    G0_d = nc.dram_tensor("G0", [128, NCH, TO], f32, kind="ExternalInput")
    OUT_d = nc.dram_tensor("OUT", [NCH, 128, TO], f32, kind="ExternalOutput")

    RG = [[0, 1, 2, 3], [4, 5, 6, 7]]

    with tile.TileContext(nc) as tc:
        with (
            tc.tile_pool(name="big", bufs=1) as big,
            tc.tile_pool(name="ps", bufs=1, space="PSUM") as psp,
            tc.tile_pool(name="dram", bufs=1, space="DRAM") as dram,
        ):
            AT = big.tile([128, CB, KC, CBW], bf16, name="ATs", tag="ATs")
            H2 = big.tile([128, KC, TO], bf16, name="H2s", tag="H2s")
            # gathered w, piece-major: piece p slot s=(j*4+q) holds global
            # k-chunk j*8 + p*4 + q   (j = source rank in group, q in 0..3)
            W = big.tile([128, NP, 4 * NB, TO], bf16, name="Ws", tag="Ws")
            H1 = big.tile([128, NCH, TO], f32, name="H1s", tag="H1s")
            G0 = big.tile([128, NCH, TO], f32, name="G0s", tag="G0s")
            WS = big.tile([128, NCH, TO], bf16, name="WSs", tag="WSs")
            OS = big.tile([128, NCH, TO], f32, name="OSs", tag="OSs")

            w_in = [dram.tile([2 * CBW, TO], bf16, name=f"w_in{p}",
                              tag=f"w_in{p}") for p in range(NP)]
            w_all = [dram.tile([NB * 2 * CBW, TO], bf16, name=f"w_all{p}",
                               tag=f"w_all{p}") for p in range(NP)]

            # ---- input DMA streams on the SP ring (no waits -> FIFO ok) ----
            # contiguous per-partition blocks; first col-block split smaller
            # so the first matmul starts early
            nc.sync.dma_start(H2[:, 0:16], H2_d[:, 0:16])
            nc.sync.dma_start(AT[:, 0, 0:8], AT_d[:, 0, 0:8])
            nc.sync.dma_start(AT[:, 0, 8:16], AT_d[:, 0, 8:16])
            nc.sync.dma_start(H2[:, 16:32], H2_d[:, 16:32])
            nc.sync.dma_start(AT[:, 0, 16:32], AT_d[:, 0, 16:32])
            nc.sync.dma_start(H1[:, 0:2], H1_d[:, 0:2])
            for cb in range(1, CB):
                nc.sync.dma_start(AT[:, cb, 0:16], AT_d[:, cb, 0:16])
                nc.sync.dma_start(AT[:, cb, 16:32], AT_d[:, cb, 16:32])
                nc.sync.dma_start(H1[:, 2 * cb:2 * cb + 2],
                                  H1_d[:, 2 * cb:2 * cb + 2])
            # G0 + gathered-w loads ride the ACT ring (so a W load waiting on
            # a collective can't head-block later input loads)
            nc.scalar.dma_start(G0[:], G0_d[:])

            # ---- pass 1 (col-block-major) + 2-piece AllGather pipeline ----
            ps1 = [psp.tile([128, TO], f32, name=f"y{n}", tag=f"bank{n}")
                   for n in range(NCH)]
            for cb in range(CB):
                with nc.named_scope(f"p1cb{cb}"):
                    for k in range(KC):
                        for i in range(2):
                            n = 2 * cb + i
                            nc.tensor.matmul(
                                ps1[n][:],
                                AT[:, cb, k, i * 128:(i + 1) * 128],
                                H2[:, k],
                                start=(k == 0),
                                stop=(k == KC - 1),
                            )
                    for i in range(2):
                        n = 2 * cb + i
                        nc.vector.tensor_add(WS[:, n], ps1[n][:], H1[:, n])
                if cb % 2 == 1:
                    p = cb // 2
                    # SWDGE: off the input-load HWDGE rings, and on the
                    # engine that triggers the collective.
                    nc.gpsimd.dma_start(
                        w_in[p][:].rearrange("(c p) n -> p c n", p=128),
                        WS[:, 4 * p:4 * p + 4])
                    nc.gpsimd.collective_compute(
                        "AllGather",
                        mybir.AluOpType.bypass,
                        replica_groups=RG,
                        ins=[w_in[p][:]],
                        outs=[w_all[p][:]],
                    )
                    nc.scalar.dma_start(
                        W[:, p],
                        w_all[p][:].rearrange("(s p) n -> p s n", p=128))

            # ---- pass 2: accumulate pieces in gather order ----
            # piece p slot s=(j*4+q) holds global k-chunk j*8 + p*4 + q;
            # lhsT for out n-chunk n lives at AT[:, n//2, k, (n%2)*128:...]
            ps2 = [psp.tile([128, TO], f32, name=f"o{n}", tag=f"bank{n}")
                   for n in range(NCH)]

            def lhsT(kg, n):
                return AT[:, n // 2, kg, (n % 2) * 128:(n % 2 + 1) * 128]

            def kglobal(p, s):
                return (s // 4) * (KC // NB) + p * 4 + (s % 4)

            with nc.named_scope("p2p0"):
                for s in range(4 * NB):
                    kg = kglobal(0, s)
                    for n in range(NCH):
                        nc.tensor.matmul(
                            ps2[n][:], lhsT(kg, n), W[:, 0, s],
                            start=(s == 0), stop=False)
            # last piece n-outer so each out chunk finishes early ->
            # epilogue (add g0, relu, store) pipelines with remaining matmuls
            Relu = mybir.ActivationFunctionType.Relu
            with nc.named_scope("p2last"):
                for n in range(NCH):
                    for s in range(4 * NB):
                        kg = kglobal(1, s)
                        nc.tensor.matmul(
                            ps2[n][:], lhsT(kg, n), W[:, 1, s],
                            start=False, stop=(s == 4 * NB - 1))
                    nc.vector.tensor_add(OS[:, n], ps2[n][:], G0[:, n])
                    nc.scalar.activation(OS[:, n], OS[:, n], Relu)
                    if n % 2 == 1:
                        nc.sync.dma_start(
                            OUT_d[n - 1:n + 1].rearrange("c p n -> p c n"),
                            OS[:, n - 1:n + 1])

    nc.compile()
    return nc


def _get_nc():
    if "nc" not in _CACHE:
        _CACHE["nc"] = _build_nc()
    return _CACHE["nc"]


def _prepare_in_maps(X, A_q, Theta1, bias):
    bf16 = ml_dtypes.bfloat16
    X = np.asarray(X, dtype=np.float32)
    A_q = np.asarray(A_q, dtype=np.float32)
    Theta1 = np.asarray(Theta1, dtype=np.float32)
    bias = np.asarray(bias, dtype=np.float32)

    Th = Theta1.reshape(F, K, O)
    Th0, Th1, Th2 = Th[:, 0], Th[:, 1], Th[:, 2]

    in_maps = []
    for b in range(B):
        Xb = X[b]                                   # (T, N, F)
        # [n, (t, o)] node-major layouts
        h2 = np.transpose(2.0 * (Xb @ Th2), (1, 0, 2)).reshape(N, TO)
        h1 = np.transpose(Xb @ Th1, (1, 0, 2)).reshape(N, TO)
        g0 = np.transpose(Xb @ (Th0 - Th2) + bias, (1, 0, 2)).reshape(N, TO)
        # partition-major: [128, KC, TO]
        h2b = np.ascontiguousarray(
            h2.reshape(KC, 128, TO).transpose(1, 0, 2)).astype(bf16)
        for j in range(NB):
            rows = slice(j * RS, (j + 1) * RS)
            # (4096, 1024) -> [128, cb, k, 256]
            ATj = A_q[b, rows].T.astype(bf16)
            ATj = np.ascontiguousarray(
                ATj.reshape(KC, 128, CB, CBW).transpose(1, 2, 0, 3))
            in_maps.append({
                "AT": ATj,
                "H2": h2b,
                "H1": np.ascontiguousarray(
                    h1[rows].reshape(NCH, 128, TO).transpose(1, 0, 2)),
                "G0": np.ascontiguousarray(
                    g0[rows].reshape(NCH, 128, TO).transpose(1, 0, 2)),
            })
    return in_maps


def run_with_results(inputs, **spmd_kwargs):
    """Returns (full_output, BassKernelResults). spmd_kwargs forwarded to
    run_bass_kernel_spmd (e.g. trace=True)."""
    from concourse.bass_utils import run_bass_kernel_spmd

    nc = _get_nc()
    in_maps = _prepare_in_maps(**inputs)
    res = run_bass_kernel_spmd(
        nc, in_maps, core_ids=list(range(NCORES)), **spmd_kwargs)

    out = np.empty((B, T, N, O), dtype=np.float32)
    for c in range(NCORES):
        b, j = divmod(c, NB)
        blk = res.results[c]["OUT"].reshape(RS, T, O)   # [n, t, o]
        out[b, :, j * RS:(j + 1) * RS, :] = np.transpose(blk, (1, 0, 2))
    return out, res


def kernel(X, A_q, Theta1, bias):
    out, _ = run_with_results(
        {"X": X, "A_q": A_q, "Theta1": Theta1, "bias": bias})
    return out


# revision 6
# speedup vs baseline: 1.4169x; 1.1288x over previous
"""Trainium2 Bass kernel for nn_D_GCN (Chebyshev-style GCN diffusion).

Reference computation (per batch b):
    x0 = X                       (T, N, F) node features
    x1 = A x0                    (diffusion over nodes)
    x2 = 2 A x1 - x0
    out = relu(stack_k(x_k) @ Theta1 + bias)     Theta row index = f*K + k

Algebraic refactoring used here (Theta_k := Theta1[k::3]):
    out = relu( g0 + A @ (h1 + A @ h2) )
    g0  = x0 (Theta_0 - Theta_2) + bias    [host, f32]
    h1  = x0 Theta_1                       [host, f32]
    h2  = 2 x0 Theta_2                     [host, bf16]
All feature-dim matmuls (2% of FLOPs) are folded to host preprocessing;
the device runs the two dense N x N diffusion matmuls (98% of FLOPs,
all of A's traffic) as plain chained matmuls with no transposes.

Sharding: 8 cores = 2 batches x 4 node-blocks of 1024 rows.
Each core holds A^T[:, block] (bf16, SBUF-resident across both passes),
computes y = A_blk @ h2 for its rows, w = h1 + y, AllGathers w within
its 4-core batch group, then out_blk = relu(A_blk @ w + g0).

Pipelining: pass 1 runs column-block-major (4 blocks of 256 out rows),
w is AllGathered in 2 pieces as soon as each half is ready, and pass 2
consumes the gathered pieces in order so the second gather overlaps the
first piece's matmuls. All dense inputs are laid out partition-major on
the host so each DMA moves large contiguous per-partition blocks.
"""

import sys

if "/opt/trn_rl_repo" not in sys.path:
    sys.path.insert(0, "/opt/trn_rl_repo")

import numpy as np
import ml_dtypes

B, T, N, F, O = 2, 8, 4096, 32, 32
K = 3
NCORES = 8
NB = 4             # node blocks (shards) per batch
RS = N // NB       # rows per shard = 1024
NCH = RS // 128    # 8 n-chunks per shard
KC = N // 128      # 32 k-chunks (contraction)
TO = T * O         # 256 free columns
CB = 4             # pass-1 column blocks per shard (256 out rows each)
CBW = RS // CB
NP = 2             # AllGather pieces (2 column blocks each)

_CACHE = {}


def _build_nc():
    import concourse.mybir as mybir
    import concourse.tile as tile
    from concourse import bacc

    f32 = mybir.dt.float32
    bf16 = mybir.dt.bfloat16
    fp8 = mybir.dt.float8e4

    nc = bacc.Bacc(None, num_devices=NCORES)

    # all inputs partition-major: leading dim 128 = SBUF partition
    AT_d = nc.dram_tensor("AT", [128, CB, KC, CBW], fp8, kind="ExternalInput")
    H2_d = nc.dram_tensor("H2", [128, KC, TO], fp8, kind="ExternalInput")
    H1_d = nc.dram_tensor("H1", [128, NCH, TO], f32, kind="ExternalInput")
    G0_d = nc.dram_tensor("G0", [128, NCH, TO], f32, kind="ExternalInput")
    OUT_d = nc.dram_tensor("OUT", [NCH, 128, TO], f32, kind="ExternalOutput")

    RG = [[0, 1, 2, 3], [4, 5, 6, 7]]

    with tile.TileContext(nc) as tc:
        with (
            tc.tile_pool(name="big", bufs=1) as big,
            tc.tile_pool(name="ps", bufs=1, space="PSUM") as psp,
            tc.tile_pool(name="dram", bufs=1, space="DRAM") as dram,
        ):
            AT = big.tile([128, CB, KC, CBW], fp8, name="ATs", tag="ATs")
            H2 = big.tile([128, KC, TO], fp8, name="H2s", tag="H2s")
            # gathered w, piece-major: piece p slot s=(j*4+q) holds global
            # k-chunk j*8 + p*4 + q   (j = source rank in group, q in 0..3)
            W = big.tile([128, NP, 4 * NB, TO], fp8, name="Ws", tag="Ws")
            H1 = big.tile([128, NCH, TO], f32, name="H1s", tag="H1s")
            G0 = big.tile([128, NCH, TO], f32, name="G0s", tag="G0s")
            WS = big.tile([128, NCH, TO], fp8, name="WSs", tag="WSs")
            OS = big.tile([128, NCH, TO], f32, name="OSs", tag="OSs")

            w_in = [dram.tile([2 * CBW, TO], fp8, name=f"w_in{p}",
                              tag=f"w_in{p}") for p in range(NP)]
            w_all = [dram.tile([NB * 2 * CBW, TO], fp8, name=f"w_all{p}",
                               tag=f"w_all{p}") for p in range(NP)]

            # ---- input DMA streams on the SP ring (no waits -> FIFO ok) ----
            # contiguous per-partition blocks; first col-block split smaller
            # so the first matmul starts early
            nc.sync.dma_start(H2[:, 0:16], H2_d[:, 0:16])
            nc.sync.dma_start(AT[:, 0, 0:8], AT_d[:, 0, 0:8])
            nc.sync.dma_start(AT[:, 0, 8:16], AT_d[:, 0, 8:16])
            nc.sync.dma_start(H2[:, 16:32], H2_d[:, 16:32])
            nc.sync.dma_start(AT[:, 0, 16:32], AT_d[:, 0, 16:32])
            nc.sync.dma_start(H1[:, 0:2], H1_d[:, 0:2])
            for cb in range(1, CB):
                nc.sync.dma_start(AT[:, cb, 0:16], AT_d[:, cb, 0:16])
                nc.sync.dma_start(AT[:, cb, 16:32], AT_d[:, cb, 16:32])
                nc.sync.dma_start(H1[:, 2 * cb:2 * cb + 2],
                                  H1_d[:, 2 * cb:2 * cb + 2])
            # G0 + gathered-w loads ride the ACT ring (so a W load waiting on
            # a collective can't head-block later input loads)
            nc.scalar.dma_start(G0[:], G0_d[:])

            # ---- pass 1 (col-block-major) + 2-piece AllGather pipeline ----
            ps1 = [psp.tile([128, TO], f32, name=f"y{n}", tag=f"bank{n}")
                   for n in range(NCH)]
            for cb in range(CB):
                with nc.named_scope(f"p1cb{cb}"):
                    for kp in range(KC // 2):
                        for i in range(2):
                            n = 2 * cb + i
                            nc.tensor.matmul(
                                ps1[n][:],
                                AT[:, cb, 2 * kp:2 * kp + 2,
                                   i * 128:(i + 1) * 128],
                                H2[:, 2 * kp:2 * kp + 2],
                                start=(kp == 0),
                                stop=(kp == KC // 2 - 1),
                                perf_mode=mybir.MatmulPerfMode.DoubleRow,
                            )
                    for i in range(2):
                        n = 2 * cb + i
                        # w*16 = h1*16 + psum/(SCALE_A/16)  (psum = SCALE_A*y)
                        nc.vector.scalar_tensor_tensor(
                            WS[:, n], ps1[n][:], 1.0 / 256.0, H1[:, n],
                            mybir.AluOpType.mult, mybir.AluOpType.add)
                if cb % 2 == 1:
                    p = cb // 2
                    # SWDGE: off the input-load HWDGE rings, and on the
                    # engine that triggers the collective.
                    nc.gpsimd.dma_start(
                        w_in[p][:].rearrange("(c p) n -> p c n", p=128),
                        WS[:, 4 * p:4 * p + 4])
                    nc.gpsimd.collective_compute(
                        "AllGather",
                        mybir.AluOpType.bypass,
                        replica_groups=RG,
                        ins=[w_in[p][:]],
                        outs=[w_all[p][:]],
                    )
                    nc.scalar.dma_start(
                        W[:, p],
                        w_all[p][:].rearrange("(s p) n -> p s n", p=128))

            # ---- pass 2: accumulate pieces in gather order ----
            # piece p slot s=(j*4+q) holds global k-chunk j*8 + p*4 + q;
            # lhsT for out n-chunk n lives at AT[:, n//2, k, (n%2)*128:...]
            ps2 = [psp.tile([128, TO], f32, name=f"o{n}", tag=f"bank{n}")
                   for n in range(NCH)]

            def lhsT2(kg, n):
                return AT[:, n // 2, kg:kg + 2,
                          (n % 2) * 128:(n % 2 + 1) * 128]

            def kglobal(p, s):
                return (s // 4) * (KC // NB) + p * 4 + (s % 4)

            with nc.named_scope("p2p0"):
                for sp in range(2 * NB):
                    s0 = 2 * sp
                    kg = kglobal(0, s0)
                    for n in range(NCH):
                        nc.tensor.matmul(
                            ps2[n][:], lhsT2(kg, n), W[:, 0, s0:s0 + 2],
                            start=(sp == 0), stop=False,
                            perf_mode=mybir.MatmulPerfMode.DoubleRow)
            # last piece n-outer so each out chunk finishes early ->
            # epilogue (add g0, relu, store) pipelines with remaining matmuls
            Relu = mybir.ActivationFunctionType.Relu
            with nc.named_scope("p2last"):
                for n in range(NCH):
                    for sp in range(2 * NB):
                        s0 = 2 * sp
                        kg = kglobal(1, s0)
                        nc.tensor.matmul(
                            ps2[n][:], lhsT2(kg, n), W[:, 1, s0:s0 + 2],
                            start=False, stop=(sp == 2 * NB - 1),
                            perf_mode=mybir.MatmulPerfMode.DoubleRow)
                    # out = psum/(SCALE_A*SCALE_W) + g0
                    nc.vector.scalar_tensor_tensor(
                        OS[:, n], ps2[n][:], 1.0 / 65536.0, G0[:, n],
                        mybir.AluOpType.mult, mybir.AluOpType.add)
                    nc.scalar.activation(OS[:, n], OS[:, n], Relu)
                    if n % 2 == 1:
                        nc.sync.dma_start(
                            OUT_d[n - 1:n + 1].rearrange("c p n -> p c n"),
                            OS[:, n - 1:n + 1])

    nc.compile()
    return nc


def _get_nc():
    if "nc" not in _CACHE:
        _CACHE["nc"] = _build_nc()
    return _CACHE["nc"]


SCALE_A = 4096.0
SCALE_W = 16.0


def _prepare_in_maps(X, A_q, Theta1, bias):
    fp8 = ml_dtypes.float8_e4m3
    X = np.asarray(X, dtype=np.float32)
    A_q = np.asarray(A_q, dtype=np.float32)
    Theta1 = np.asarray(Theta1, dtype=np.float32)
    bias = np.asarray(bias, dtype=np.float32)

    Th = Theta1.reshape(F, K, O)
    Th0, Th1, Th2 = Th[:, 0], Th[:, 1], Th[:, 2]

    in_maps = []
    for b in range(B):
        Xb = X[b]                                   # (T, N, F)
        # [n, (t, o)] node-major layouts
        h2 = np.transpose(2.0 * (Xb @ Th2), (1, 0, 2)).reshape(N, TO)
        h1 = np.transpose(Xb @ Th1, (1, 0, 2)).reshape(N, TO)
        g0 = np.transpose(Xb @ (Th0 - Th2) + bias, (1, 0, 2)).reshape(N, TO)
        # partition-major: [128, KC, TO]
        h2b = np.ascontiguousarray(
            h2.reshape(KC, 128, TO).transpose(1, 0, 2)).astype(fp8)
        for j in range(NB):
            rows = slice(j * RS, (j + 1) * RS)
            # (4096, 1024) -> [128, cb, k, 256]
            ATj = (A_q[b, rows].T * SCALE_A).astype(fp8)
            ATj = np.ascontiguousarray(
                ATj.reshape(KC, 128, CB, CBW).transpose(1, 2, 0, 3))
            in_maps.append({
                "AT": ATj,
                "H2": h2b,
                "H1": np.ascontiguousarray(
                    (SCALE_W * h1[rows]).reshape(NCH, 128, TO)
                    .transpose(1, 0, 2)),
                "G0": np.ascontiguousarray(
                    g0[rows].reshape(NCH, 128, TO).transpose(1, 0, 2)),
            })
    return in_maps


def run_with_results(inputs, **spmd_kwargs):
    """Returns (full_output, BassKernelResults). spmd_kwargs forwarded to
    run_bass_kernel_spmd (e.g. trace=True)."""
    from concourse.bass_utils import run_bass_kernel_spmd

    nc = _get_nc()
    in_maps = _prepare_in_maps(**inputs)
    res = run_bass_kernel_spmd(
        nc, in_maps, core_ids=list(range(NCORES)), **spmd_kwargs)

    out = np.empty((B, T, N, O), dtype=np.float32)
    for c in range(NCORES):
        b, j = divmod(c, NB)
        blk = res.results[c]["OUT"].reshape(RS, T, O)   # [n, t, o]
        out[b, :, j * RS:(j + 1) * RS, :] = np.transpose(blk, (1, 0, 2))
    return out, res


def kernel(X, A_q, Theta1, bias):
    out, _ = run_with_results(
        {"X": X, "A_q": A_q, "Theta1": Theta1, "bias": bias})
    return out


# revision 8
# speedup vs baseline: 1.6626x; 1.1734x over previous
"""Trainium2 Bass kernel for nn_D_GCN (Chebyshev-style GCN diffusion).

Reference computation (per batch b):
    x0 = X                       (T, N, F) node features
    x1 = A x0                    (diffusion over nodes)
    x2 = 2 A x1 - x0
    out = relu(stack_k(x_k) @ Theta1 + bias)     Theta row index = f*K + k

Algebraic refactoring (Theta_k := Theta1[k::3]):
    out = relu( g0 + A @ (h1 + A @ h2) )
    g0  = x0 (Theta_0 - Theta_2) + bias    [host, f32]
    h1  = x0 Theta_1                       [host, bf16, x16]
    h2  = 2 x0 Theta_2                     [host, fp8]
All feature-dim matmuls (2% of FLOPs) fold into host preprocessing; the
device runs the two dense N x N diffusion matmuls as fp8 DoubleRow
matmuls (A scaled by 4096 into e4m3 range, w scaled by 16; exact f32
g0 carries the dominant output term, so fp8 on the small diffusion
terms costs ~1e-3 relative error).

Sharding: 8 cores = 2 batches x 4 node-blocks of 1024 rows. Measured on
this runtime, any collective pays a ~70 us first-op barrier per
execution, so instead of AllGathering the intermediate w each core
redundantly computes the FULL w = h1 + A h2 for its batch (pass 1,
replicated 4x within the batch group - the PE would otherwise idle on
the barrier), then computes its own 1024-row output block in pass 2.
Zero collectives, zero cross-core dependencies.

Per-core contraction order is "my 8 k-chunks first, then the rest"
(slot order), applied consistently by the host to A's rows, h2, h1 and
pass-1 output rows, so the SPMD program indexes everything uniformly:
 - A2 (resident, 4 MiB fp8): A^T[slot rows, my 1024 cols] - serves as
   pass-1 lhsT for my 4 column blocks AND pass-2 lhsT.
 - A1 (streamed, 12 MiB fp8): A^T[slot rows, other 12 col blocks].
All inputs are partition-major so every DMA moves large contiguous
per-partition blocks.
"""

import sys

if "/opt/trn_rl_repo" not in sys.path:
    sys.path.insert(0, "/opt/trn_rl_repo")

import numpy as np
import ml_dtypes

B, T, N, F, O = 2, 8, 4096, 32, 32
K = 3
NCORES = 8
NB = 4             # node blocks (shards) per batch
RS = N // NB       # rows per shard = 1024
NCH = RS // 128    # 8 n-chunks per shard
KC = N // 128      # 32 k-chunks (contraction)
TO = T * O         # 256 free columns
CBW = 256          # pass-1 column-block width
NCB = N // CBW     # 16 column blocks total (4 mine + 12 streamed)

SCALE_A = 4096.0
SCALE_W = 16.0

_CACHE = {}


def _build_nc():
    import concourse.mybir as mybir
    import concourse.tile as tile
    from concourse import bacc

    f32 = mybir.dt.float32
    bf16 = mybir.dt.bfloat16
    fp8 = mybir.dt.float8e4
    DR = mybir.MatmulPerfMode.DoubleRow

    nc = bacc.Bacc(None, num_devices=NCORES)

    # partition-major inputs; contraction (k) dim in per-core slot order
    A2_d = nc.dram_tensor("A2", [128, KC, RS], fp8, kind="ExternalInput")
    A1_d = nc.dram_tensor("A1", [NCB - NB, 128, KC, CBW], fp8,
                          kind="ExternalInput")
    H2_d = nc.dram_tensor("H2", [128, KC, TO], fp8, kind="ExternalInput")
    H1_d = nc.dram_tensor("H1", [128, KC, TO], bf16, kind="ExternalInput")
    G0_d = nc.dram_tensor("G0", [128, NCH, TO], f32, kind="ExternalInput")
    OUT_d = nc.dram_tensor("OUT", [NCH, 128, TO], f32, kind="ExternalOutput")

    with tile.TileContext(nc) as tc:
        with (
            tc.tile_pool(name="big", bufs=1) as big,
            tc.tile_pool(name="ablk", bufs=4) as ablk,
            tc.tile_pool(name="ps", bufs=1, space="PSUM") as psp,
        ):
            A2 = big.tile([128, KC, RS], fp8, name="A2s", tag="A2s")
            H2 = big.tile([128, KC, TO], fp8, name="H2s", tag="H2s")
            H1 = big.tile([128, KC, TO], bf16, name="H1s", tag="H1s")
            G0 = big.tile([128, NCH, TO], f32, name="G0s", tag="G0s")
            WS = big.tile([128, KC, TO], fp8, name="WSs", tag="WSs")
            OS = big.tile([128, NCH, TO], f32, name="OSs", tag="OSs")

            # ---- input streams (SP ring: stream blocks; ACT ring: rest) ----
            nc.sync.dma_start(H2[:, 0:16], H2_d[:, 0:16])
            nc.sync.dma_start(H2[:, 16:32], H2_d[:, 16:32])
            ablk_tiles = []
            for sb in range(NCB - NB):
                t = ablk.tile([128, KC, CBW], fp8, name=f"ab{sb}", tag="ab")
                nc.sync.dma_start(t[:], A1_d[sb])
                ablk_tiles.append(t)
                if sb == 1:
                    nc.scalar.dma_start(H1[:, 0:16], H1_d[:, 0:16])
                if sb == 3:
                    nc.scalar.dma_start(H1[:, 16:32], H1_d[:, 16:32])
                if sb == 5:
                    nc.scalar.dma_start(A2[:, :, 0:512],
                                        A2_d[:, :, 0:512])
                if sb == 7:
                    nc.scalar.dma_start(A2[:, :, 512:1024],
                                        A2_d[:, :, 512:1024])
                if sb == 9:
                    nc.scalar.dma_start(G0[:], G0_d[:])

            # ---- pass 1: w = h1 + A h2 for ALL slot rows ----
            # streamed blocks first (slots 8..31), then my blocks (0..7)
            # psum banks rotate; STT drains each block to WS (fp8, x16)
            def p1_block(c0, lhs_of):
                """compute w chunks c0, c0+1 (slot-row chunks)"""
                tiles = []
                for i in range(2):
                    pst = psp.tile([128, TO], f32, name=f"y{(c0 + i) % 8}",
                                   tag=f"bank{(c0 + i) % 8}")
                    for kp in range(KC // 2):
                        nc.tensor.matmul(
                            pst[:], lhs_of(kp, i), H2[:, 2 * kp:2 * kp + 2],
                            start=(kp == 0), stop=(kp == KC // 2 - 1),
                            perf_mode=DR)
                    tiles.append(pst)
                for i in range(2):
                    # w*16 = h1*16 + psum*(16/4096)
                    nc.vector.scalar_tensor_tensor(
                        WS[:, c0 + i], tiles[i][:], 1.0 / 256.0, H1[:, c0 + i],
                        mybir.AluOpType.mult, mybir.AluOpType.add)

            with nc.named_scope("pass1"):
                for sb in range(NCB - NB):
                    t = ablk_tiles[sb]
                    p1_block(
                        2 * NB + 2 * sb,
                        lambda kp, i, t=t: t[:, 2 * kp:2 * kp + 2,
                                             i * 128:(i + 1) * 128])
                for cb in range(NB):
                    p1_block(
                        2 * cb,
                        lambda kp, i, cb=cb: A2[:, 2 * kp:2 * kp + 2,
                                                cb * CBW + i * 128:
                                                cb * CBW + (i + 1) * 128])

            # ---- pass 2: out rows = relu(A2^T w + g0), n-outer ----
            Relu = mybir.ActivationFunctionType.Relu
            with nc.named_scope("pass2"):
                for n in range(NCH):
                    pst = psp.tile([128, TO], f32, name=f"o{n}",
                                   tag=f"bank{n}")
                    for sp in range(KC // 2):
                        nc.tensor.matmul(
                            pst[:],
                            A2[:, 2 * sp:2 * sp + 2,
                               n * 128:(n + 1) * 128],
                            WS[:, 2 * sp:2 * sp + 2],
                            start=(sp == 0), stop=(sp == KC // 2 - 1),
                            perf_mode=DR)
                    # out = psum/(SCALE_A*SCALE_W) + g0
                    nc.vector.scalar_tensor_tensor(
                        OS[:, n], pst[:], 1.0 / 65536.0, G0[:, n],
                        mybir.AluOpType.mult, mybir.AluOpType.add)
                    nc.scalar.activation(OS[:, n], OS[:, n], Relu)
                    if n % 2 == 1:
                        nc.sync.dma_start(
                            OUT_d[n - 1:n + 1].rearrange("c p n -> p c n"),
                            OS[:, n - 1:n + 1])

    nc.compile()
    return nc


def _get_nc():
    if "nc" not in _CACHE:
        _CACHE["nc"] = _build_nc()
    return _CACHE["nc"]


def _prepare_in_maps(X, A_q, Theta1, bias):
    fp8 = ml_dtypes.float8_e4m3
    bf16 = ml_dtypes.bfloat16
    X = np.asarray(X, dtype=np.float32)
    A_q = np.asarray(A_q, dtype=np.float32)
    Theta1 = np.asarray(Theta1, dtype=np.float32)
    bias = np.asarray(bias, dtype=np.float32)

    Th = Theta1.reshape(F, K, O)
    Th0, Th1, Th2 = Th[:, 0], Th[:, 1], Th[:, 2]

    in_maps = []
    for b in range(B):
        Xb = X[b]                                   # (T, N, F)
        # [n, (t, o)] node-major layouts
        h2 = np.transpose(2.0 * (Xb @ Th2), (1, 0, 2)).reshape(N, TO)
        h1 = np.transpose(Xb @ Th1, (1, 0, 2)).reshape(N, TO)
        g0 = np.transpose(Xb @ (Th0 - Th2) + bias, (1, 0, 2)).reshape(N, TO)
        AT = (A_q[b].T * SCALE_A).astype(fp8)       # [m, n] scaled
        for j in range(NB):
            my = slice(j * RS, (j + 1) * RS)
            # slot order: my 8 k-chunks first, then the others
            order = np.r_[np.arange(j * RS, (j + 1) * RS),
                          np.arange(0, j * RS), np.arange((j + 1) * RS, N)]
            ATs = AT[order]                          # [slot rows, n]
            A2 = np.ascontiguousarray(
                ATs[:, my].reshape(KC, 128, RS).transpose(1, 0, 2))
            # other column blocks, in stream order (all except my 4)
            other_cols = np.r_[np.arange(0, j * RS),
                               np.arange((j + 1) * RS, N)]
            A1 = np.ascontiguousarray(
                ATs[:, other_cols].reshape(KC, 128, NCB - NB, CBW)
                .transpose(2, 1, 0, 3))              # [blk, 128, KC, CBW]
            h2s = np.ascontiguousarray(
                h2[order].reshape(KC, 128, TO).transpose(1, 0, 2)).astype(fp8)
            h1s = np.ascontiguousarray(
                (SCALE_W * h1[order]).reshape(KC, 128, TO)
                .transpose(1, 0, 2)).astype(bf16)
            in_maps.append({
                "A2": A2,
                "A1": A1,
                "H2": h2s,
                "H1": h1s,
                "G0": np.ascontiguousarray(
                    g0[my].reshape(NCH, 128, TO).transpose(1, 0, 2)),
            })
    return in_maps


def run_with_results(inputs, **spmd_kwargs):
    """Returns (full_output, BassKernelResults). spmd_kwargs forwarded to
    run_bass_kernel_spmd (e.g. trace=True)."""
    from concourse.bass_utils import run_bass_kernel_spmd

    nc = _get_nc()
    in_maps = _prepare_in_maps(**inputs)
    res = run_bass_kernel_spmd(
        nc, in_maps, core_ids=list(range(NCORES)), **spmd_kwargs)

    out = np.empty((B, T, N, O), dtype=np.float32)
    for c in range(NCORES):
        b, j = divmod(c, NB)
        blk = res.results[c]["OUT"].reshape(RS, T, O)   # [n, t, o]
        out[b, :, j * RS:(j + 1) * RS, :] = np.transpose(blk, (1, 0, 2))
    return out, res


def kernel(X, A_q, Theta1, bias):
    out, _ = run_with_results(
        {"X": X, "A_q": A_q, "Theta1": Theta1, "bias": bias})
    return out


# revision 9
# speedup vs baseline: 1.6930x; 1.0183x over previous
"""Trainium2 Bass kernel for nn_D_GCN (Chebyshev-style GCN diffusion).

Reference computation (per batch b):
    x0 = X                       (T, N, F) node features
    x1 = A x0                    (diffusion over nodes)
    x2 = 2 A x1 - x0
    out = relu(stack_k(x_k) @ Theta1 + bias)     Theta row index = f*K + k

Algebraic refactoring (Theta_k := Theta1[k::3]):
    out = relu( g0 + A @ (h1 + A @ h2) )
    g0  = x0 (Theta_0 - Theta_2) + bias    [host, f32]
    h1  = x0 Theta_1                       [host, bf16, x16]
    h2  = 2 x0 Theta_2                     [host, fp8]
All feature-dim matmuls (2% of FLOPs) fold into host preprocessing; the
device runs the two dense N x N diffusion matmuls as fp8 DoubleRow
matmuls (A scaled by 4096 into e4m3 range, w scaled by 16; exact f32
g0 carries the dominant output term, so fp8 on the small diffusion
terms costs ~1e-3 relative error).

Sharding: 8 cores = 2 batches x 4 node-blocks of 1024 rows. Measured on
this runtime, any collective pays a ~70 us first-op barrier per
execution, so instead of AllGathering the intermediate w each core
redundantly computes the FULL w = h1 + A h2 for its batch (pass 1,
replicated 4x within the batch group - the PE would otherwise idle on
the barrier), then computes its own 1024-row output block in pass 2.
Zero collectives, zero cross-core dependencies.

Per-core contraction order is "my 8 k-chunks first, then the rest"
(slot order), applied consistently by the host to A's rows, h2, h1 and
pass-1 output rows, so the SPMD program indexes everything uniformly:
 - A2 (resident, 4 MiB fp8): A^T[slot rows, my 1024 cols] - serves as
   pass-1 lhsT for my 4 column blocks AND pass-2 lhsT.
 - A1 (streamed, 12 MiB fp8): A^T[slot rows, other 12 col blocks].
All inputs are partition-major so every DMA moves large contiguous
per-partition blocks.
"""

import sys

if "/opt/trn_rl_repo" not in sys.path:
    sys.path.insert(0, "/opt/trn_rl_repo")

import numpy as np
import ml_dtypes

B, T, N, F, O = 2, 8, 4096, 32, 32
K = 3
NCORES = 8
NB = 4             # node blocks (shards) per batch
RS = N // NB       # rows per shard = 1024
NCH = RS // 128    # 8 n-chunks per shard
KC = N // 128      # 32 k-chunks (contraction)
TO = T * O         # 256 free columns
CBW = 256          # pass-1 column-block width
NCB = N // CBW     # 16 column blocks total (4 mine + 12 streamed)

SCALE_A = 4096.0
SCALE_W = 16.0

_CACHE = {}


def _build_nc():
    import concourse.mybir as mybir
    import concourse.tile as tile
    from concourse import bacc

    f32 = mybir.dt.float32
    bf16 = mybir.dt.bfloat16
    fp8 = mybir.dt.float8e4
    DR = mybir.MatmulPerfMode.DoubleRow

    nc = bacc.Bacc(None, num_devices=NCORES)

    # partition-major inputs; contraction (k) dim in per-core slot order
    A2_d = nc.dram_tensor("A2", [128, KC, RS], fp8, kind="ExternalInput")
    A1_d = nc.dram_tensor("A1", [NCB - NB, 128, KC, CBW], fp8,
                          kind="ExternalInput")
    H2_d = nc.dram_tensor("H2", [128, KC, TO], fp8, kind="ExternalInput")
    H1_d = nc.dram_tensor("H1", [128, KC, TO], fp8, kind="ExternalInput")
    G0_d = nc.dram_tensor("G0", [128, NCH, TO], f32, kind="ExternalInput")
    OUT_d = nc.dram_tensor("OUT", [NCH, 128, TO], f32, kind="ExternalOutput")

    with tile.TileContext(nc) as tc:
        with (
            tc.tile_pool(name="big", bufs=1) as big,
            tc.tile_pool(name="ablk", bufs=6) as ablk,
            tc.tile_pool(name="ps", bufs=1, space="PSUM") as psp,
        ):
            A2 = big.tile([128, KC, RS], fp8, name="A2s", tag="A2s")
            H2 = big.tile([128, KC, TO], fp8, name="H2s", tag="H2s")
            H1 = big.tile([128, KC, TO], fp8, name="H1s", tag="H1s")
            G0 = big.tile([128, NCH, TO], f32, name="G0s", tag="G0s")
            WS = big.tile([128, KC, TO], fp8, name="WSs", tag="WSs")
            OS = big.tile([128, NCH, TO], f32, name="OSs", tag="OSs")

            # ---- one explicitly-ordered input stream on the SP ring ----
            # (a second ring would contend for HBM exactly when the first
            # stream block is needed; FIFO order IS the prefetch schedule)
            ablk_tiles = [
                ablk.tile([128, KC, CBW], fp8, name=f"ab{sb}", tag="ab")
                for sb in range(NCB - NB)
            ]

            def load_ab(sb):
                nc.sync.dma_start(ablk_tiles[sb][:], A1_d[sb])

            load_ab(0)
            nc.sync.dma_start(H2[:, 0:4], H2_d[:, 0:4])
            nc.sync.dma_start(H2[:, 4:32], H2_d[:, 4:32])
            load_ab(1)
            load_ab(2)
            nc.sync.dma_start(H1[:, 0:16], H1_d[:, 0:16])
            load_ab(3)
            load_ab(4)
            nc.sync.dma_start(H1[:, 16:32], H1_d[:, 16:32])
            load_ab(5)
            nc.sync.dma_start(A2[:, 0:16], A2_d[:, 0:16])
            load_ab(6)
            load_ab(7)
            nc.sync.dma_start(A2[:, 16:32], A2_d[:, 16:32])
            load_ab(8)
            nc.sync.dma_start(G0[:], G0_d[:])
            load_ab(9)
            load_ab(10)
            load_ab(11)

            # ---- pass 1: w = h1 + A h2 for ALL slot rows ----
            # streamed blocks first (slots 8..31), then my blocks (0..7)
            # psum banks rotate; STT drains each block to WS (fp8, x16)
            def p1_block(c0, lhs_of):
                """compute w chunks c0, c0+1 (slot-row chunks)"""
                tiles = []
                for i in range(2):
                    pst = psp.tile([128, TO], f32, name=f"y{(c0 + i) % 8}",
                                   tag=f"bank{(c0 + i) % 8}")
                    for kp in range(KC // 2):
                        nc.tensor.matmul(
                            pst[:], lhs_of(kp, i), H2[:, 2 * kp:2 * kp + 2],
                            start=(kp == 0), stop=(kp == KC // 2 - 1),
                            perf_mode=DR)
                    tiles.append(pst)
                for i in range(2):
                    # w*16 = h1*16 + psum*(16/4096)
                    nc.vector.scalar_tensor_tensor(
                        WS[:, c0 + i], tiles[i][:], 1.0 / 256.0, H1[:, c0 + i],
                        mybir.AluOpType.mult, mybir.AluOpType.add)

            with nc.named_scope("pass1"):
                for sb in range(NCB - NB):
                    t = ablk_tiles[sb]
                    p1_block(
                        2 * NB + 2 * sb,
                        lambda kp, i, t=t: t[:, 2 * kp:2 * kp + 2,
                                             i * 128:(i + 1) * 128])
                for cb in range(NB):
                    p1_block(
                        2 * cb,
                        lambda kp, i, cb=cb: A2[:, 2 * kp:2 * kp + 2,
                                                cb * CBW + i * 128:
                                                cb * CBW + (i + 1) * 128])

            # ---- pass 2: out rows = relu(A2^T w + g0), n-outer ----
            Relu = mybir.ActivationFunctionType.Relu
            with nc.named_scope("pass2"):
                for n in range(NCH):
                    pst = psp.tile([128, TO], f32, name=f"o{n}",
                                   tag=f"bank{n}")
                    for sp in range(KC // 2):
                        nc.tensor.matmul(
                            pst[:],
                            A2[:, 2 * sp:2 * sp + 2,
                               n * 128:(n + 1) * 128],
                            WS[:, 2 * sp:2 * sp + 2],
                            start=(sp == 0), stop=(sp == KC // 2 - 1),
                            perf_mode=DR)
                    # out = psum/(SCALE_A*SCALE_W) + g0
                    nc.vector.scalar_tensor_tensor(
                        OS[:, n], pst[:], 1.0 / 65536.0, G0[:, n],
                        mybir.AluOpType.mult, mybir.AluOpType.add)
                    nc.scalar.activation(OS[:, n], OS[:, n], Relu)
                    if n % 2 == 1:
                        nc.scalar.dma_start(
                            OUT_d[n - 1:n + 1].rearrange("c p n -> p c n"),
                            OS[:, n - 1:n + 1])

    nc.compile()
    return nc


def _get_nc():
    if "nc" not in _CACHE:
        _CACHE["nc"] = _build_nc()
    return _CACHE["nc"]


def _prepare_in_maps(X, A_q, Theta1, bias):
    fp8 = ml_dtypes.float8_e4m3
    bf16 = ml_dtypes.bfloat16
    X = np.asarray(X, dtype=np.float32)
    A_q = np.asarray(A_q, dtype=np.float32)
    Theta1 = np.asarray(Theta1, dtype=np.float32)
    bias = np.asarray(bias, dtype=np.float32)

    Th = Theta1.reshape(F, K, O)
    Th0, Th1, Th2 = Th[:, 0], Th[:, 1], Th[:, 2]

    in_maps = []
    for b in range(B):
        Xb = X[b]                                   # (T, N, F)
        # [n, (t, o)] node-major layouts
        h2 = np.transpose(2.0 * (Xb @ Th2), (1, 0, 2)).reshape(N, TO)
        h1 = np.transpose(Xb @ Th1, (1, 0, 2)).reshape(N, TO)
        g0 = np.transpose(Xb @ (Th0 - Th2) + bias, (1, 0, 2)).reshape(N, TO)
        AT = (A_q[b].T * SCALE_A).astype(fp8)       # [m, n] scaled
        for j in range(NB):
            my = slice(j * RS, (j + 1) * RS)
            # slot order: my 8 k-chunks first, then the others
            order = np.r_[np.arange(j * RS, (j + 1) * RS),
                          np.arange(0, j * RS), np.arange((j + 1) * RS, N)]
            ATs = AT[order]                          # [slot rows, n]
            A2 = np.ascontiguousarray(
                ATs[:, my].reshape(KC, 128, RS).transpose(1, 0, 2))
            # other column blocks, in stream order (all except my 4)
            other_cols = np.r_[np.arange(0, j * RS),
                               np.arange((j + 1) * RS, N)]
            A1 = np.ascontiguousarray(
                ATs[:, other_cols].reshape(KC, 128, NCB - NB, CBW)
                .transpose(2, 1, 0, 3))              # [blk, 128, KC, CBW]
            h2s = np.ascontiguousarray(
                h2[order].reshape(KC, 128, TO).transpose(1, 0, 2)).astype(fp8)
            h1s = np.ascontiguousarray(
                (SCALE_W * h1[order]).reshape(KC, 128, TO)
                .transpose(1, 0, 2)).astype(fp8)
            in_maps.append({
                "A2": A2,
                "A1": A1,
                "H2": h2s,
                "H1": h1s,
                "G0": np.ascontiguousarray(
                    g0[my].reshape(NCH, 128, TO).transpose(1, 0, 2)),
            })
    return in_maps


def run_with_results(inputs, **spmd_kwargs):
    """Returns (full_output, BassKernelResults). spmd_kwargs forwarded to
    run_bass_kernel_spmd (e.g. trace=True)."""
    from concourse.bass_utils import run_bass_kernel_spmd

    nc = _get_nc()
    in_maps = _prepare_in_maps(**inputs)
    res = run_bass_kernel_spmd(
        nc, in_maps, core_ids=list(range(NCORES)), **spmd_kwargs)

    out = np.empty((B, T, N, O), dtype=np.float32)
    for c in range(NCORES):
        b, j = divmod(c, NB)
        blk = res.results[c]["OUT"].reshape(RS, T, O)   # [n, t, o]
        out[b, :, j * RS:(j + 1) * RS, :] = np.transpose(blk, (1, 0, 2))
    return out, res


def kernel(X, A_q, Theta1, bias):
    out, _ = run_with_results(
        {"X": X, "A_q": A_q, "Theta1": Theta1, "bias": bias})
    return out


# revision 10
# speedup vs baseline: 1.8326x; 1.0824x over previous
"""Trainium2 Bass kernel for nn_D_GCN (Chebyshev-style GCN diffusion).

Reference computation (per batch b):
    x0 = X                       (T, N, F) node features
    x1 = A x0                    (diffusion over nodes)
    x2 = 2 A x1 - x0
    out = relu(stack_k(x_k) @ Theta1 + bias)     Theta row index = f*K + k

Algebraic refactoring (Theta_k := Theta1[k::3]):
    out = relu( g0 + A @ (h1 + A @ h2) )
    g0  = x0 (Theta_0 - Theta_2) + bias    [host, f32]
    h1  = x0 Theta_1                       [host, bf16, x16]
    h2  = 2 x0 Theta_2                     [host, fp8]
All feature-dim matmuls (2% of FLOPs) fold into host preprocessing; the
device runs the two dense N x N diffusion matmuls as fp8 DoubleRow
matmuls (A scaled by 4096 into e4m3 range, w scaled by 16; exact f32
g0 carries the dominant output term, so fp8 on the small diffusion
terms costs ~1e-3 relative error).

Sharding: 8 cores = 2 batches x 4 node-blocks of 1024 rows. Measured on
this runtime, any collective pays a ~70 us first-op barrier per
execution, so instead of AllGathering the intermediate w each core
redundantly computes the FULL w = h1 + A h2 for its batch (pass 1,
replicated 4x within the batch group - the PE would otherwise idle on
the barrier), then computes its own 1024-row output block in pass 2.
Zero collectives, zero cross-core dependencies.

Per-core contraction order is "my 8 k-chunks first, then the rest"
(slot order), applied consistently by the host to A's rows, h2, h1 and
pass-1 output rows, so the SPMD program indexes everything uniformly:
 - A2 (resident, 4 MiB fp8): A^T[slot rows, my 1024 cols] - serves as
   pass-1 lhsT for my 4 column blocks AND pass-2 lhsT.
 - A1 (streamed, 12 MiB fp8): A^T[slot rows, other 12 col blocks].
All inputs are partition-major so every DMA moves large contiguous
per-partition blocks.
"""

import sys

if "/opt/trn_rl_repo" not in sys.path:
    sys.path.insert(0, "/opt/trn_rl_repo")

import numpy as np
import ml_dtypes

B, T, N, F, O = 2, 8, 4096, 32, 32
K = 3
NCORES = 8
NB = 4             # node blocks (shards) per batch
RS = N // NB       # rows per shard = 1024
NCH = RS // 128    # 8 n-chunks per shard
KC = N // 128      # 32 k-chunks (contraction)
TO = T * O         # 256 free columns
CBW = 256          # pass-1 column-block width
NCB = N // CBW     # 16 column blocks total (4 mine + 12 streamed)

SCALE_A = 4096.0
SCALE_W = 16.0

_CACHE = {}


def _build_nc():
    import concourse.mybir as mybir
    import concourse.tile as tile
    from concourse import bacc

    f32 = mybir.dt.float32
    bf16 = mybir.dt.bfloat16
    fp8 = mybir.dt.float8e4
    DR = mybir.MatmulPerfMode.DoubleRow

    nc = bacc.Bacc(None, num_devices=NCORES)

    # partition-major inputs; contraction (k) dim in per-core slot order
    A2_d = nc.dram_tensor("A2", [128, KC, RS], fp8, kind="ExternalInput")
    A1_d = nc.dram_tensor("A1", [NCB - NB, 128, KC, CBW], fp8,
                          kind="ExternalInput")
    H2_d = nc.dram_tensor("H2", [128, KC, TO], fp8, kind="ExternalInput")
    H1_d = nc.dram_tensor("H1", [128, KC, TO], bf16, kind="ExternalInput")
    G0_d = nc.dram_tensor("G0", [128, NCH, TO], f32, kind="ExternalInput")
    OUT_d = nc.dram_tensor("OUT", [NCH, 128, TO], f32, kind="ExternalOutput")

    with tile.TileContext(nc) as tc:
        with (
            tc.tile_pool(name="big", bufs=1) as big,
            tc.tile_pool(name="ablk", bufs=8) as ablk,
            tc.tile_pool(name="ps", bufs=1, space="PSUM") as psp,
        ):
            A2 = big.tile([128, KC, RS], fp8, name="A2s", tag="A2s")
            H2 = big.tile([128, KC, TO], fp8, name="H2s", tag="H2s")
            H1 = big.tile([128, KC, TO], bf16, name="H1s", tag="H1s")
            G0 = big.tile([128, NCH, TO], f32, name="G0s", tag="G0s")
            WS = big.tile([128, KC, TO], fp8, name="WSs", tag="WSs")
            OS = big.tile([128, NCH, TO], f32, name="OSs", tag="OSs")

            # ---- one explicitly-ordered input stream on the SP ring ----
            # (a second ring would contend for HBM exactly when the first
            # stream block is needed; FIFO order IS the prefetch schedule)
            ablk_tiles = [
                ablk.tile([128, KC, CBW], fp8, name=f"ab{sb}", tag="ab")
                for sb in range(NCB - NB)
            ]

            def load_ab(sb):
                nc.sync.dma_start(ablk_tiles[sb][:], A1_d[sb])

            load_ab(0)
            nc.sync.dma_start(H2[:, 0:4], H2_d[:, 0:4])
            nc.sync.dma_start(H2[:, 4:32], H2_d[:, 4:32])
            load_ab(1)
            load_ab(2)
            nc.sync.dma_start(H1[:, 0:16], H1_d[:, 0:16])
            load_ab(3)
            load_ab(4)
            nc.sync.dma_start(H1[:, 16:32], H1_d[:, 16:32])
            load_ab(5)
            load_ab(6)
            nc.sync.dma_start(A2[:, 0:16], A2_d[:, 0:16])
            load_ab(7)
            load_ab(8)
            nc.sync.dma_start(A2[:, 16:32], A2_d[:, 16:32])
            load_ab(9)
            load_ab(10)
            load_ab(11)
            nc.sync.dma_start(G0[:], G0_d[:])

            # ---- pass 1: w = h1 + A h2 for ALL slot rows ----
            # streamed blocks first (slots 8..31), then my blocks (0..7)
            # psum banks rotate; STT drains each block to WS (fp8, x16)
            def p1_block(c0, lhs_of):
                """compute w chunks c0, c0+1 (slot-row chunks)"""
                tiles = []
                for i in range(2):
                    pst = psp.tile([128, TO], f32, name=f"y{(c0 + i) % 8}",
                                   tag=f"bank{(c0 + i) % 8}")
                    for kp in range(KC // 2):
                        nc.tensor.matmul(
                            pst[:], lhs_of(kp, i), H2[:, 2 * kp:2 * kp + 2],
                            start=(kp == 0), stop=(kp == KC // 2 - 1),
                            perf_mode=DR)
                    tiles.append(pst)
                for i in range(2):
                    # w*16 = h1*16 + psum*(16/4096)
                    nc.vector.scalar_tensor_tensor(
                        WS[:, c0 + i], tiles[i][:], 1.0 / 256.0, H1[:, c0 + i],
                        mybir.AluOpType.mult, mybir.AluOpType.add)

            with nc.named_scope("pass1"):
                for sb in range(NCB - NB):
                    t = ablk_tiles[sb]
                    p1_block(
                        2 * NB + 2 * sb,
                        lambda kp, i, t=t: t[:, 2 * kp:2 * kp + 2,
                                             i * 128:(i + 1) * 128])
                for cb in range(NB):
                    p1_block(
                        2 * cb,
                        lambda kp, i, cb=cb: A2[:, 2 * kp:2 * kp + 2,
                                                cb * CBW + i * 128:
                                                cb * CBW + (i + 1) * 128])

            # ---- pass 2: out rows = relu(A2^T w + g0), n-outer ----
            Relu = mybir.ActivationFunctionType.Relu
            with nc.named_scope("pass2"):
                for n in range(NCH):
                    pst = psp.tile([128, TO], f32, name=f"o{n}",
                                   tag=f"bank{n}")
                    for sp in range(KC // 2):
                        nc.tensor.matmul(
                            pst[:],
                            A2[:, 2 * sp:2 * sp + 2,
                               n * 128:(n + 1) * 128],
                            WS[:, 2 * sp:2 * sp + 2],
                            start=(sp == 0), stop=(sp == KC // 2 - 1),
                            perf_mode=DR)
                    # out = psum/(SCALE_A*SCALE_W) + g0
                    nc.vector.scalar_tensor_tensor(
                        OS[:, n], pst[:], 1.0 / 65536.0, G0[:, n],
                        mybir.AluOpType.mult, mybir.AluOpType.add)
                    nc.scalar.activation(OS[:, n], OS[:, n], Relu)
                    if n % 2 == 1:
                        nc.scalar.dma_start(
                            OUT_d[n - 1:n + 1].rearrange("c p n -> p c n"),
                            OS[:, n - 1:n + 1])

    nc.compile()
    return nc


def _get_nc():
    if "nc" not in _CACHE:
        _CACHE["nc"] = _build_nc()
    return _CACHE["nc"]


def _prepare_in_maps(X, A_q, Theta1, bias):
    fp8 = ml_dtypes.float8_e4m3
    bf16 = ml_dtypes.bfloat16
    X = np.asarray(X, dtype=np.float32)
    A_q = np.asarray(A_q, dtype=np.float32)
    Theta1 = np.asarray(Theta1, dtype=np.float32)
    bias = np.asarray(bias, dtype=np.float32)

    Th = Theta1.reshape(F, K, O)
    Th0, Th1, Th2 = Th[:, 0], Th[:, 1], Th[:, 2]

    in_maps = []
    for b in range(B):
        Xb = X[b]                                   # (T, N, F)
        # [n, (t, o)] node-major layouts
        h2 = np.transpose(2.0 * (Xb @ Th2), (1, 0, 2)).reshape(N, TO)
        h1 = np.transpose(Xb @ Th1, (1, 0, 2)).reshape(N, TO)
        g0 = np.transpose(Xb @ (Th0 - Th2) + bias, (1, 0, 2)).reshape(N, TO)
        AT = (A_q[b].T * SCALE_A).astype(fp8)       # [m, n] scaled
        for j in range(NB):
            my = slice(j * RS, (j + 1) * RS)
            # slot order: my 8 k-chunks first, then the others
            order = np.r_[np.arange(j * RS, (j + 1) * RS),
                          np.arange(0, j * RS), np.arange((j + 1) * RS, N)]
            ATs = AT[order]                          # [slot rows, n]
            A2 = np.ascontiguousarray(
                ATs[:, my].reshape(KC, 128, RS).transpose(1, 0, 2))
            # other column blocks, in stream order (all except my 4)
            other_cols = np.r_[np.arange(0, j * RS),
                               np.arange((j + 1) * RS, N)]
            A1 = np.ascontiguousarray(
                ATs[:, other_cols].reshape(KC, 128, NCB - NB, CBW)
                .transpose(2, 1, 0, 3))              # [blk, 128, KC, CBW]
            h2s = np.ascontiguousarray(
                h2[order].reshape(KC, 128, TO).transpose(1, 0, 2)).astype(fp8)
            h1s = np.ascontiguousarray(
                (SCALE_W * h1[order]).reshape(KC, 128, TO)
                .transpose(1, 0, 2)).astype(bf16)
            in_maps.append({
                "A2": A2,
                "A1": A1,
                "H2": h2s,
                "H1": h1s,
                "G0": np.ascontiguousarray(
                    g0[my].reshape(NCH, 128, TO).transpose(1, 0, 2)),
            })
    return in_maps


def run_with_results(inputs, **spmd_kwargs):
    """Returns (full_output, BassKernelResults). spmd_kwargs forwarded to
    run_bass_kernel_spmd (e.g. trace=True)."""
    from concourse.bass_utils import run_bass_kernel_spmd

    nc = _get_nc()
    in_maps = _prepare_in_maps(**inputs)
    res = run_bass_kernel_spmd(
        nc, in_maps, core_ids=list(range(NCORES)), **spmd_kwargs)

    out = np.empty((B, T, N, O), dtype=np.float32)
    for c in range(NCORES):
        b, j = divmod(c, NB)
        blk = res.results[c]["OUT"].reshape(RS, T, O)   # [n, t, o]
        out[b, :, j * RS:(j + 1) * RS, :] = np.transpose(blk, (1, 0, 2))
    return out, res


def kernel(X, A_q, Theta1, bias):
    out, _ = run_with_results(
        {"X": X, "A_q": A_q, "Theta1": Theta1, "bias": bias})
    return out
